# revision 18
# baseline (speedup 1.0000x reference)
"""LoRA Linear kernel for Trainium2, 8 NeuronCores, data-parallel over tokens.

out = x @ W^T + bias + 2.0 * (x @ A^T) @ B^T
  x: [4, 2048, 4096] f32, W: [4096, 4096], bias: [4096], A: [16, 4096], B: [4096, 16]

Strategy:
  - Fold the LoRA update into the weight on the host: W' = W + 2.0 * (B @ A)
    (mathematically identical), so the device runs a single dense GEMM + bias.
  - Flatten tokens (8192) and shard 1024 tokens per core (pure data parallel,
    no collectives; gather on host).
  - bf16 operands: same 1 cycle/row PE speed as f32r but half the HBM traffic
    (rel err ~2e-3 vs the 2e-2 gate). PSUM accumulation stays f32.
  - x^T blocks are the stationary operand, W'^T slices the moving one, so the
    output lands untransposed as [tokens, features].
  - The contraction dim d maps to partitions p-major (d = p*32 + ko), which
    makes every DMA's per-partition HBM runs 4-8KB contiguous (the ko-major
    mapping gives 1-2KB runs and ~half DMA throughput). wt is additionally
    host-blocked per o-chunk so its slices are contiguous too.
  - Pass structure over 8 o-chunks of 512, with 8 PSUM banks:
      pass 0: k-outer (for k: for mt) so the PE consumes x^T chunks in DMA
        arrival order - compute starts ~10us in instead of stalling ~40us
        for the full x^T load.
      pass 1: k-outer with a skewed (anti-diagonal) entry, so each PSUM bank
        is first touched only after pass 0's staggered DVE drain frees it.
      passes 2-7: mt-outer/k-inner, so banks close 6.9us apart and the DVE
        bias-add drain fully overlaps compute (no pass-boundary PE stall,
        which would also re-throttle the HAM clock gate).
  - Bias is added by the (otherwise idle) DVE during PSUM->SBUF copy-out.
  - A few zero matmuls up front warm the PE's HAM clock gate (1.2 -> 2.4 GHz)
    while the first DMA chunks are still in flight.
"""

import sys
from contextlib import ExitStack

import numpy as np
import ml_dtypes

sys.path.insert(0, "/opt/trn_rl_repo")

import concourse.bacc as bacc  # noqa: E402
import concourse.bass as bass  # noqa: E402
import concourse.mybir as mybir  # noqa: E402
import concourse.tile as tile  # noqa: E402
from concourse.bass import ts  # noqa: E402
from concourse.bass_utils import run_bass_kernel_spmd  # noqa: E402

P = 128
B_DIM, S_DIM = 4, 2048
D = 4096          # in_features (contraction)
O = 4096          # out_features
SCALING = 2.0     # alpha / rank = 32/16
NCORES = 8
M = (B_DIM * S_DIM) // NCORES   # tokens per core = 1024
KD = D // P       # 32 contraction tiles
OC = 512          # out-feature chunk (moving free dim per matmul)
NOC = O // OC     # 8 o-chunks
NMT = M // P      # 8 token tiles
NWARM = 12        # HAM warm-up matmuls (bridge the PE to first-data arrival)

BF16 = mybir.dt.bfloat16
F32 = mybir.dt.float32


def pass_order(pass_idx: int):
    """(k, mt) emission order for one o-chunk pass."""
    if pass_idx == 0:
        return [(k, mt) for k in range(KD) for mt in range(NMT)]
    if pass_idx == 1:
        # anti-diagonal: bank mt first touched at diagonal mt, matching the
        # rate at which pass 0's DVE drain frees banks
        order = []
        for s in range(KD + NMT - 1):
            for mt in range(NMT):
                k = s - mt
                if 0 <= k < KD:
                    order.append((k, mt))
        return order
    return [(k, mt) for mt in range(NMT) for k in range(KD)]


def build_program() -> bass.Bass:
    # Bacc (not plain Bass): its compile() pipeline splits multi-wait
    # matmuls via event semaphores.
    nc = bacc.Bacc()
    xt = nc.dram_tensor("xt", [D, M], BF16, kind="ExternalInput")
    # host-blocked: [noc*128, 32, 512]; rows (oc*128+p) hold d=p*32+ko
    wt = nc.dram_tensor("wt", [NOC * P, KD, OC], BF16, kind="ExternalInput")
    biasb = nc.dram_tensor("biasb", [P, O], F32, kind="ExternalInput")
    out = nc.dram_tensor("out", [M, O], F32, kind="ExternalOutput")

    xt_r = xt.rearrange("(p ko) m -> p ko m", ko=KD)   # [128, 32, 1024]

    with ExitStack() as ctx:
        tc = ctx.enter_context(tile.TileContext(nc))
        xt_pool = ctx.enter_context(tc.tile_pool(name="xtp", bufs=1))
        cpool = ctx.enter_context(tc.tile_pool(name="cpool", bufs=1))
        wt_pool = ctx.enter_context(tc.tile_pool(name="wtp", bufs=2))
        bias_pool = ctx.enter_context(tc.tile_pool(name="biasp", bufs=2))
        out_pool = ctx.enter_context(tc.tile_pool(name="outp", bufs=8))
        ps_pool = ctx.enter_context(tc.tile_pool(name="psp", bufs=1, space="PSUM"))

        xt_sb = xt_pool.tile([P, KD, M], BF16)       # 64 KB/partition
        wmA = cpool.tile([P, P], BF16)
        wmB = cpool.tile([P, OC], BF16)
        ps = [ps_pool.tile([P, OC], F32, name=f"ps_{i}") for i in range(NMT)]

        # HAM warm-up: zero matmuls keep the PE busy (and un-throttled)
        # while the first real DMA chunks land. Memsets go on the vector
        # queue so the gpsimd ring can start posting x^T chunks immediately.
        nc.vector.memset(wmA[:], 0.0)
        nc.vector.memset(wmB[:], 0.0)
        for w in range(NWARM):
            nc.tensor.matmul(
                ps[w % NMT][:], lhsT=wmA[:], rhs=wmB[:], start=True, stop=True
            )

        # x^T k-chunks alternate between the scalar and gpsimd rings; the
        # k-outer pass-0 loop consumes them in arrival order. The first two
        # chunks are single k-slices so the PE can start under the 8-core
        # HBM contention burst at kernel start.
        xsplits = [1, 1] + [2] * 15
        k0 = 0
        for h, xk in enumerate(xsplits):
            eng = nc.scalar if h % 2 == 0 else nc.gpsimd
            eng.dma_start(
                xt_sb[:, k0 : k0 + xk, :], xt_r[:, k0 : k0 + xk, :]
            )
            k0 += xk

        for oc in range(NOC):
            wt_sb = wt_pool.tile([P, KD, OC], BF16)  # 32 KB/partition
            # pass 0 consumes wt in k-order as it streams, so split finely
            # (leading chunks smallest); later passes are prefetched a full
            # pass ahead - coarser chunks mean fewer PE wait-events (each
            # satisfied wait still breaks back-to-back matmul pipelining)
            if oc == 0:
                wsplits = [2, 2, 4, 4, 4, 4, 4, 4, 4]
            elif oc == 1:
                wsplits = [4] * 8
            else:
                wsplits = [16, 16]
            k0 = 0
            for wk in wsplits:
                nc.sync.dma_start(
                    wt_sb[:, k0 : k0 + wk, :], wt[ts(oc, P), k0 : k0 + wk, :]
                )
                k0 += wk
            # per-pass bias slice: keeps the contended startup DMA window
            # free of the 2MB bias block (only needed at each pass's end)
            bias_sb = bias_pool.tile([P, OC], F32, name="bias_t")
            nc.gpsimd.dma_start(bias_sb[:], biasb[:, ts(oc, OC)])
            for k, mt in pass_order(oc):
                nc.tensor.matmul(
                    ps[mt][:],
                    lhsT=xt_sb[:, k, ts(mt, P)],
                    rhs=wt_sb[:, k, :],
                    start=(k == 0),
                    stop=(k == KD - 1),
                )
            for mt in range(NMT):
                ot = out_pool.tile([P, OC], F32)
                nc.vector.tensor_tensor(
                    ot[:], ps[mt][:], bias_sb[:], mybir.AluOpType.add
                )
                # spread the final pass's drain across all three rings
                if oc == NOC - 1:
                    eng = (nc.gpsimd, nc.scalar, nc.sync)[mt % 3]
                else:
                    eng = nc.gpsimd
                eng.dma_start(out[ts(mt, P), ts(oc, OC)], ot[:])
    nc.compile()
    return nc


def prepare_in_maps(inputs, weight, bias, lora_a, lora_b):
    x = np.ascontiguousarray(np.asarray(inputs, dtype=np.float32)).reshape(
        B_DIM * S_DIM, D
    )
    w_folded = np.asarray(weight, dtype=np.float32) + SCALING * (
        np.asarray(lora_b, dtype=np.float32) @ np.asarray(lora_a, dtype=np.float32)
    )
    # [D, O] -> [NOC, P, KD, OC] with d = p*KD + ko, then flatten the first two
    wt = np.ascontiguousarray(
        w_folded.T.reshape(P, KD, NOC, OC).transpose(2, 0, 1, 3).reshape(
            NOC * P, KD, OC
        )
    ).astype(ml_dtypes.bfloat16)
    biasb = np.ascontiguousarray(
        np.tile(np.asarray(bias, dtype=np.float32)[None, :], (P, 1))
    )
    in_maps = []
    for c in range(NCORES):
        xt_c = np.ascontiguousarray(x[c * M : (c + 1) * M].T).astype(
            ml_dtypes.bfloat16
        )
        in_maps.append({"xt": xt_c, "wt": wt, "biasb": biasb})
    return in_maps


def run(inputs, weight, bias, lora_a, lora_b, trace=False):
    nc = build_program()
    in_maps = prepare_in_maps(inputs, weight, bias, lora_a, lora_b)
    res = run_bass_kernel_spmd(nc, in_maps, list(range(NCORES)), trace=trace)
    shards = [np.asarray(res.results[c]["out"]) for c in range(NCORES)]
    out = np.concatenate(shards, axis=0).reshape(B_DIM, S_DIM, O)
    return np.ascontiguousarray(out, dtype=np.float32), res


def kernel(inputs, weight, bias, lora_a, lora_b):
    out, _ = run(inputs, weight, bias, lora_a, lora_b, trace=False)
    return out
